# revision 5
# baseline (speedup 1.0000x reference)
"""MoE (B=4,T=2048,C=1024,H=4096,E=8,top2) Trainium2 kernel.

Strategy: data-parallel over tokens (1024 tokens per core, 8 cores).
Each core, fully on-device:
  1. fp32 router (matmul + softmax) over its 1024 tokens, top-2 via max8,
     per-expert rank compaction (triangular-matmul cumsum + indirect scatter).
  2. Per expert: indirect-gather its tokens (capacity CAP, zero-padded),
     bf16 FFN fc1 -> exact GELU -> fc2, gate-scale, indirect scatter-ADD
     into the core's output shard. Padded lanes carry gate 0 -> add 0.
Host only splits/concats shards and reduces the 8-element aux partials.
"""

import numpy as np
import ml_dtypes

B, T, C, H, E, TOPK = 4, 2048, 1024, 4096, 8, 2
NCORES = 8
TOK = (B * T) // NCORES          # 1024 tokens per core
P = 128
NT = TOK // P                    # 8 token tiles per core
CC = C // P                      # 8 contraction chunks
HC = H // P                      # 32 h chunks
CAP = 384                        # per-core per-expert token capacity
NTILE = CAP // P                 # 3 tiles per expert
NBF = ml_dtypes.bfloat16

_compiled = {}


def _build():
    import concourse.bass as bass
    import concourse.bacc as bacc
    import concourse.mybir as mybir
    from concourse.tile import TileContext
    from concourse.masks import make_identity

    f32 = mybir.dt.float32
    bf16 = mybir.dt.bfloat16
    i32 = mybir.dt.int32
    AF = mybir.ActivationFunctionType
    OP = mybir.AluOpType

    nc = bacc.Bacc()

    xt_d = nc.declare_dram_parameter("xt", [C, TOK], f32, isOutput=False)
    xb_d = nc.declare_dram_parameter("xb", [TOK, C], bf16, isOutput=False)
    wgt_d = nc.declare_dram_parameter("wgt", [C, E], f32, isOutput=False)
    rb_d = nc.declare_dram_parameter("rb", [1, E], f32, isOutput=False)
    w1_d = nc.declare_dram_parameter("w1", [E, C, H], bf16, isOutput=False)
    b1_d = nc.declare_dram_parameter("b1r", [E, P, HC], f32, isOutput=False)
    w2_d = nc.declare_dram_parameter("w2", [E, H, C], bf16, isOutput=False)
    b2_d = nc.declare_dram_parameter("b2", [E, 1, C], f32, isOutput=False)

    y_d = nc.declare_dram_parameter("y", [TOK, C], f32, isOutput=True)
    imp_d = nc.declare_dram_parameter("imp", [E, 1], f32, isOutput=True)
    cnt_d = nc.declare_dram_parameter("cnt", [1, E], f32, isOutput=True)

    # combined (token_id, gate) list per expert slot, fp32
    cl_d = nc.dram_tensor("cl", [E * CAP, 2], f32)

    with TileContext(nc) as tc:
        with tc.tile_pool(name="const", bufs=1) as cpool:
            identb = cpool.tile([P, P], bf16)
            make_identity(nc, identb[:])
            # upper-triangular (inclusive) ones: value p - f, keep 0 where >0, fill 1 elsewhere
            u128 = cpool.tile([P, P], f32)
            nc.gpsimd.memset(u128[:], 0.0)
            nc.gpsimd.affine_select(
                out=u128[:], in_=u128[:], pattern=[[-1, P]],
                compare_op=OP.is_gt, fill=1.0, base=0, channel_multiplier=1)
            ones_col = cpool.tile([P, 1], f32)
            nc.vector.memset(ones_col[:], 1.0)
            ones_row = cpool.tile([1, P], f32)
            nc.vector.memset(ones_row[:], 1.0)
            eoff = cpool.tile([P, E], i32)
            nc.gpsimd.iota(eoff[:], pattern=[[CAP, E]], base=-1, channel_multiplier=0)
            eoff_f = cpool.tile([P, E], f32)
            nc.vector.tensor_copy(out=eoff_f[:], in_=eoff[:])
            rb_sb = cpool.tile([1, E], f32)
            nc.sync.dma_start(out=rb_sb[:], in_=rb_d[:])
            # zero out the slot list (garbage lanes -> token 0, gate 0)
            zf = cpool.tile([P, E * CAP * 2 // P], f32)
            nc.vector.memset(zf[:], 0.0)
            nc.sync.dma_start(
                out=cl_d.rearrange("(a b) c -> a (b c)", a=P)[:], in_=zf[:])
            base_sb = cpool.tile([1, E], f32)
            nc.vector.memset(base_sb[:], 0.0)
            # one-hot selector of partition 127 (for row extraction matmuls)
            e127 = cpool.tile([P, 1], f32)
            nc.gpsimd.memset(e127[:], 0.0)
            nc.gpsimd.affine_select(
                out=e127[:], in_=e127[:], pattern=[[0, 1]],
                compare_op=OP.not_equal, fill=1.0, base=-(P - 1),
                channel_multiplier=1)

            # ---------------- Phase 1: routing ----------------
            with tc.tile_pool(name="rt_sb", bufs=3) as rsb, \
                 tc.tile_pool(name="rt_wg", bufs=CC) as rwg, \
                 tc.tile_pool(name="rt_ps", bufs=2, space="PSUM") as rps, \
                 tc.tile_pool(name="imp_ps", bufs=1, space="PSUM") as ips:
                wg_sb = []
                for cc in range(CC):
                    wt = rwg.tile([P, E], f32, tag="wg")
                    nc.sync.dma_start(out=wt[:], in_=wgt_d[cc * P:(cc + 1) * P, :])
                    wg_sb.append(wt)
                imp_ps = ips.tile([E, 1], f32, space="PSUM")

                for ti in range(NT):
                    lg = rps.tile([P, E], f32, space="PSUM", tag="lg")
                    for cc in range(CC):
                        xtt = rsb.tile([P, P], f32, tag="xt")
                        nc.sync.dma_start(
                            out=xtt[:],
                            in_=xt_d[cc * P:(cc + 1) * P, ti * P:(ti + 1) * P])
                        nc.tensor.matmul(out=lg[:], lhsT=xtt[:], rhs=wg_sb[cc][:],
                                         start=(cc == 0), stop=False)
                    nc.tensor.matmul(out=lg[:], lhsT=ones_row[:, :P], rhs=rb_sb[:],
                                     start=False, stop=True)
                    # softmax over E (free dim)
                    mx = rsb.tile([P, 1], f32, tag="mx")
                    nc.vector.reduce_max(out=mx[:], in_=lg[:],
                                         axis=mybir.AxisListType.X)
                    nmx = rsb.tile([P, 1], f32, tag="nmx")
                    nc.vector.tensor_scalar(out=nmx[:], in0=mx[:], scalar1=-1.0,
                                            scalar2=None, op0=OP.mult)
                    ex = rsb.tile([P, E], f32, tag="ex")
                    s = rsb.tile([P, 1], f32, tag="s")
                    nc.scalar.activation(out=ex[:], in_=lg[:], func=AF.Exp,
                                         bias=nmx[:], scale=1.0, accum_out=s[:])
                    rs = rsb.tile([P, 1], f32, tag="rs")
                    nc.vector.reciprocal(out=rs[:], in_=s[:])
                    gn = rsb.tile([P, E], f32, tag="gn")
                    nc.vector.tensor_tensor(out=gn[:], in0=ex[:],
                                            in1=rs[:].to_broadcast([P, E]),
                                            op=OP.mult)
                    # importance accumulation (column sums across all tiles)
                    nc.tensor.matmul(out=imp_ps[:], lhsT=gn[:], rhs=ones_col[:],
                                     start=(ti == 0), stop=(ti == NT - 1))
                    # top2
                    t8 = rsb.tile([P, 8], f32, tag="t8")
                    nc.vector.max(out=t8[:], in_=gn[:])
                    mask2 = rsb.tile([P, E], f32, tag="mask2")
                    nc.vector.tensor_tensor(out=mask2[:], in0=gn[:],
                                            in1=t8[:, 1:2].to_broadcast([P, E]),
                                            op=OP.is_ge)
                    # within-tile cumsum + running base
                    rk = rps.tile([P, E], f32, space="PSUM", tag="rk")
                    nc.tensor.matmul(out=rk[:], lhsT=u128[:], rhs=mask2[:],
                                     start=True, stop=False)
                    nc.tensor.matmul(out=rk[:], lhsT=ones_row[:, :P], rhs=base_sb[:],
                                     start=False, stop=True)
                    # slot offsets = rank + e*CAP - 1
                    off = rsb.tile([P, E], f32, tag="off")
                    nc.vector.tensor_tensor(out=off[:], in0=rk[:], in1=eoff_f[:],
                                            op=OP.add)
                    # running totals: row 127 of off minus expert offsets
                    # (engines can't read a lone partition 127 directly)
                    ext = rps.tile([1, E], f32, space="PSUM", tag="ext")
                    nc.tensor.matmul(out=ext[:], lhsT=e127[:], rhs=off[:],
                                     start=True, stop=True)
                    nc.vector.tensor_tensor(out=base_sb[:], in0=ext[:],
                                            in1=eoff_f[0:1, :], op=OP.subtract)
                    mask1 = rsb.tile([P, E], f32, tag="mask1")
                    nc.vector.tensor_tensor(out=mask1[:], in0=gn[:],
                                            in1=t8[:, 0:1].to_broadcast([P, E]),
                                            op=OP.is_ge)
                    m2x = rsb.tile([P, E], f32, tag="m2x")
                    nc.vector.tensor_sub(out=m2x[:], in0=mask2[:], in1=mask1[:])
                    val = rsb.tile([P, 2], f32, tag="val")
                    nc.gpsimd.iota(val[:, 0:1], pattern=[[0, 1]], base=ti * P,
                                   channel_multiplier=1,
                                   allow_small_or_imprecise_dtypes=True)
                    for mk, gcol in ((mask1, t8[:, 0:1]), (m2x, t8[:, 1:2])):
                        tmp = rsb.tile([P, E], f32, tag="tmp")
                        nc.vector.tensor_tensor(out=tmp[:], in0=mk[:], in1=off[:],
                                                op=OP.mult)
                        of = rsb.tile([P, 1], f32, tag="of")
                        nc.vector.reduce_sum(out=of[:], in_=tmp[:],
                                             axis=mybir.AxisListType.X)
                        oi = rsb.tile([P, 1], i32, tag="oi")
                        nc.vector.tensor_copy(out=oi[:], in_=of[:])
                        nc.vector.tensor_copy(out=val[:, 1:2], in_=gcol)
                        nc.gpsimd.indirect_dma_start(
                            out=cl_d[:], out_offset=bass.IndirectOffsetOnAxis(
                                ap=oi[:, :1], axis=0),
                            in_=val[:], in_offset=None,
                            bounds_check=E * CAP - 1, oob_is_err=False)
                cnt_sb = rsb.tile([1, E], f32, tag="cnt")
                nc.vector.tensor_copy(out=cnt_sb[:], in_=base_sb[:])
                nc.sync.dma_start(out=cnt_d[:], in_=cnt_sb[:])
                imp_sb = rsb.tile([E, 1], f32, tag="imps")
                nc.vector.tensor_copy(out=imp_sb[:], in_=imp_ps[:])
                nc.sync.dma_start(out=imp_d[:], in_=imp_sb[:])

            # ---------------- Phase 2: expert FFN ----------------
            with tc.tile_pool(name="w1p", bufs=CC + 2) as w1p, \
                 tc.tile_pool(name="w2p", bufs=HC + 2) as w2p, \
                 tc.tile_pool(name="bp", bufs=2) as bp, \
                 tc.tile_pool(name="act", bufs=2) as act, \
                 tc.tile_pool(name="xgp", bufs=2) as xgp, \
                 tc.tile_pool(name="tp_ps", bufs=2, space="PSUM") as tps, \
                 tc.tile_pool(name="h_ps", bufs=2, space="PSUM") as hps, \
                 tc.tile_pool(name="o_ps", bufs=1, space="PSUM") as ops:
                for e in range(E):
                    b1t = bp.tile([P, HC], f32, tag="b1")
                    nc.sync.dma_start(out=b1t[:], in_=b1_d[e])
                    b2t = bp.tile([1, C], f32, tag="b2")
                    nc.sync.dma_start(out=b2t[:], in_=b2_d[e])
                    w1c = []
                    for cc in range(CC):
                        wt = w1p.tile([P, H], bf16, tag="w1")
                        nc.sync.dma_start(out=wt[:],
                                          in_=w1_d[e, cc * P:(cc + 1) * P, :])
                        w1c.append(wt)
                    w2c = []
                    for hc in range(HC):
                        wt = w2p.tile([P, C], bf16, tag="w2")
                        nc.sync.dma_start(out=wt[:],
                                          in_=w2_d[e, hc * P:(hc + 1) * P, :])
                        w2c.append(wt)
                    for j in range(NTILE):
                        base = e * CAP + j * P
                        ig = xgp.tile([P, 2], f32, tag="ig")
                        nc.sync.dma_start(out=ig[:], in_=cl_d[base:base + P, :])
                        it = xgp.tile([P, 1], i32, tag="it")
                        nc.vector.tensor_copy(out=it[:], in_=ig[:, 0:1])
                        xg = xgp.tile([P, C], bf16, tag="xg")
                        nc.gpsimd.indirect_dma_start(
                            out=xg[:], out_offset=None, in_=xb_d[:],
                            in_offset=bass.IndirectOffsetOnAxis(ap=it[:, :1], axis=0),
                            bounds_check=TOK - 1, oob_is_err=False)
                        xgT = act.tile([P, C], bf16, tag="xgT")
                        for cc in range(CC):
                            tp = tps.tile([P, P], bf16, space="PSUM", tag="tp")
                            nc.tensor.transpose(out=tp[:],
                                                in_=xg[:, cc * P:(cc + 1) * P],
                                                identity=identb[:])
                            nc.vector.tensor_copy(
                                out=xgT[:, cc * P:(cc + 1) * P], in_=tp[:])
                        hT = act.tile([P, H], bf16, tag="hT")
                        for hc in range(HC):
                            hp = hps.tile([P, P], f32, space="PSUM", tag="hp")
                            for cc in range(CC):
                                nc.tensor.matmul(
                                    out=hp[:],
                                    lhsT=w1c[cc][:, hc * P:(hc + 1) * P],
                                    rhs=xgT[:, cc * P:(cc + 1) * P],
                                    start=(cc == 0), stop=(cc == CC - 1))
                            nc.scalar.activation(out=hT[:, hc * P:(hc + 1) * P],
                                                 in_=hp[:], func=AF.Gelu,
                                                 bias=b1t[:, hc:hc + 1], scale=1.0)
                        out_sb = act.tile([P, C], f32, tag="osb")
                        gt_b = ig[:, 1:2]
                        for half in range(2):
                            sl = slice(half * 512, (half + 1) * 512)
                            op2 = ops.tile([P, 512], f32, space="PSUM", tag="op")
                            for hc in range(HC):
                                nc.tensor.matmul(out=op2[:],
                                                 lhsT=hT[:, hc * P:(hc + 1) * P],
                                                 rhs=w2c[hc][:, sl],
                                                 start=(hc == 0), stop=False)
                            nc.tensor.matmul(out=op2[:], lhsT=ones_row[:, :P],
                                             rhs=b2t[:, sl], start=False, stop=True)
                            nc.vector.tensor_tensor(
                                out=out_sb[:, sl], in0=op2[:],
                                in1=gt_b.to_broadcast([P, 512]), op=OP.mult)
                        nc.gpsimd.indirect_dma_start(
                            out=y_d[:], out_offset=bass.IndirectOffsetOnAxis(
                                ap=it[:, :1], axis=0),
                            in_=out_sb[:], in_offset=None,
                            bounds_check=TOK - 1, oob_is_err=False,
                            compute_op=OP.add)

    nc.finalize()
    return nc


def kernel(x, w_gating, router_bias, fc1_weight, fc1_bias, fc2_weight, fc2_bias,
           topk):
    from concourse.bass_utils import run_bass_kernel_spmd

    assert int(topk) == TOPK
    if "nc" not in _compiled:
        _compiled["nc"] = _build()
    nc = _compiled["nc"]

    x = np.asarray(x, dtype=np.float32)
    xf = np.ascontiguousarray(x.reshape(B * T, C))
    w1b = np.ascontiguousarray(np.asarray(fc1_weight, np.float32)).astype(NBF)
    w2b = np.ascontiguousarray(np.asarray(fc2_weight, np.float32)).astype(NBF)
    b1r = np.ascontiguousarray(
        np.asarray(fc1_bias, np.float32).reshape(E, HC, P).transpose(0, 2, 1))
    b2 = np.asarray(fc2_bias, np.float32).reshape(E, 1, C)
    wgt = np.ascontiguousarray(np.asarray(w_gating, np.float32).T)
    rb = np.asarray(router_bias, np.float32).reshape(1, E)

    in_maps = []
    for c in range(NCORES):
        sh = xf[c * TOK:(c + 1) * TOK]
        in_maps.append({
            "xt": np.ascontiguousarray(sh.T),
            "xb": np.ascontiguousarray(sh).astype(NBF),
            "wgt": wgt, "rb": rb,
            "w1": w1b, "b1r": b1r, "w2": w2b, "b2": b2,
        })
    res = run_bass_kernel_spmd(nc, in_maps, list(range(NCORES)))
    _compiled["last_res"] = res

    y = np.concatenate([res.results[c]["y"] for c in range(NCORES)], axis=0)
    cnts = np.stack([res.results[c]["cnt"][0] for c in range(NCORES)])
    if cnts.max() > CAP:
        raise RuntimeError(f"expert capacity exceeded: {cnts.max()} > {CAP}")
    imp = np.sum([res.results[c]["imp"][:, 0] for c in range(NCORES)], axis=0)
    mean_imp = (imp / np.float32(B * T)).astype(np.float32)
    aux = np.float32(E) * np.sum(mean_imp * mean_imp, dtype=np.float32)
    return y.reshape(B, T, C), np.float32(aux)
